# revision 1
# baseline (speedup 1.0000x reference)
"""Trainium2 Bass kernel for the HGCA contrastive loss (nn_HGCA_10857677324785).

loss = mean over i of 0.5*(l1_i + l2_i) where
  h1 = elu(z1@W1+b1)@W2+b2 ; h2 likewise ; an, bn = l2-normalized rows
  l1_i = -log( exp(an_i.bn_i/tau) / (sum_j exp(an_i.an_j/tau)
               + sum_j exp(an_i.bn_j/tau) - e^{1/tau}) )
  l2_i symmetric with row sums of exp(bn@bn.T) and exp(bn@an.T).

Distribution: rows sharded over 8 cores. Host rolls z1/z2 per core so each
core's row block sits at local rows [0,2048). Each core computes the full
normalized projections (cheap, O(N D^2)), then its row-block of the three
N x N similarity matrices flash-style: exp row sums on ACT (fused accum),
plus per-column partial sums of exp(an@bn.T) (for l2's "between" term, which
equals column sums of the l1 "between" matrix). Host assembles the scalar
loss from O(N) partial sums.
"""

import re

import ml_dtypes
import numpy as np

import concourse.bass as bass
import concourse.tile as tile
from concourse import mybir
from concourse.bass_utils import run_bass_kernel_spmd
from concourse.masks import make_identity
from concourse.vector_clock import ScopedClock, VectorClock

N = 16384
D = 128
NCORES = 8
R = N // NCORES  # 2048 rows per core
INV_TAU = 2.0  # 1/0.5
F32 = mybir.dt.float32
BF16 = mybir.dt.bfloat16
AF = mybir.ActivationFunctionType
OP = mybir.AluOpType

# This walrus build supports at most 2 sync waits per instruction; Tile's sem
# assignment freely emits 3-11. Post-pass: hoist excess waits onto injected
# same-engine EventSemaphore fillers (engine queues are FIFO, so waits on an
# earlier filler happen-before the original instruction executes).

_MAX_WAITS = 1


def _split_waits(nc):
    for fn in nc.m.functions:
        for bb in fn.blocks:
            insts = list(bb.instructions)
            out = []
            changed = False
            for inst in insts:
                si = inst.sync_info
                w = list(si.on_wait) if si and si.on_wait else []
                if len(w) > _MAX_WAITS:
                    changed = True
                    extra, keep = w[:-_MAX_WAITS], w[-_MAX_WAITS:]
                    for i in range(0, len(extra), _MAX_WAITS):
                        f = mybir.InstEventSemaphore(
                            name=f"{inst.name}_wsplit{i}",
                            engine=inst.engine,
                            ins=[],
                            outs=[],
                            sync_info=mybir.SyncInfo(
                                on_wait=extra[i : i + _MAX_WAITS], on_update=[]
                            ),
                        )
                        out.append(f)
                    inst.sync_info = mybir.SyncInfo(
                        on_wait=keep,
                        on_update=list(si.on_update) if si.on_update else [],
                    )
                out.append(inst)
            if changed:
                bb.instructions = out


def _patched_drain_and_barrier(self, tick_clock, wait_clock):
    nc = self.nc
    drain_inst = nc.sync.drain()
    wait_clock.add_sem_waits(
        drain_inst.ins, ScopedClock({None: tick_clock.global_clock})
    )
    nc.all_engine_barrier()
    assert self.sems is not None
    popped = nc._tile_sem_poison_stack.pop()
    assert popped is self._sem_poison
    nc.clear_and_free_semaphores(list(self.sems.allocated().values()))
    nc.all_engine_barrier()
    _split_waits(nc)


tile.TileContext._drain_and_barrier = _patched_drain_and_barrier

_NC_CACHE = None
RUN_KWARGS: dict = {}
LAST_RES = None


def _build():
    nc = bass.Bass("TRN2", target_bir_lowering=False, debug=False)

    z1_d = nc.dram_tensor("z1", [N, D], BF16, kind="ExternalInput").ap()
    z2_d = nc.dram_tensor("z2", [N, D], BF16, kind="ExternalInput").ap()
    w1_d = nc.dram_tensor("w1", [D, D], BF16, kind="ExternalInput").ap()
    w2_d = nc.dram_tensor("w2", [D, D], BF16, kind="ExternalInput").ap()
    b1_d = nc.dram_tensor("b1", [D, 1], F32, kind="ExternalInput").ap()
    b2p_d = nc.dram_tensor("b2p", [D, 1], F32, kind="ExternalInput").ap()

    rs_d = [
        nc.dram_tensor(f"rs{i}", [128, 16], F32, kind="ExternalOutput").ap()
        for i in range(3)
    ]
    cs12_d = nc.dram_tensor("cs12", [1, N], F32, kind="ExternalOutput").ap()
    num_d = nc.dram_tensor("num", [1, R], F32, kind="ExternalOutput").ap()

    with tile.TileContext(nc) as tc:
        with (
            tc.tile_pool(name="persist", bufs=1) as pers,
            tc.tile_pool(name="consts", bufs=1) as consts,
        ):
            anT = pers.tile([128, N], BF16, tag="anT")
            bnT = pers.tile([128, N], BF16, tag="bnT")
            rs_sb = [
                pers.tile([128, 16], F32, tag=f"rs{i}", name=f"rs_sb{i}")
                for i in range(3)
            ]

            ident = consts.tile([128, 128], BF16, tag="ident")
            make_identity(nc, ident[:])
            ones_col_bf = consts.tile([128, 1], BF16, tag="ocb")
            nc.gpsimd.memset(ones_col_bf[:], 1.0)
            ones_col_f = consts.tile([128, 1], F32, tag="ocf")
            nc.gpsimd.memset(ones_col_f[:], 1.0)
            ones_row_f = consts.tile([1, 128], F32, tag="orf")
            nc.gpsimd.memset(ones_row_f[:], 1.0)
            w1sb = consts.tile([128, 128], BF16, tag="w1")
            nc.sync.dma_start(w1sb[:], w1_d[:])
            w2sb = consts.tile([128, 128], BF16, tag="w2")
            nc.sync.dma_start(w2sb[:], w2_d[:])
            b1sb = consts.tile([128, 1], F32, tag="b1")
            nc.sync.dma_start(b1sb[:], b1_d[:])
            b2psb = consts.tile([128, 1], F32, tag="b2p")
            nc.sync.dma_start(b2psb[:], b2p_d[:])

            # ---------------- setup: projections + normalize ----------------
            with (
                tc.tile_pool(name="szt", bufs=2) as szt,
                tc.tile_pool(name="sw", bufs=4) as sw,
                tc.tile_pool(name="sp2", bufs=2, space="PSUM") as sp2,
                tc.tile_pool(name="sp1", bufs=1, space="PSUM") as sp1,
            ):
                for t, (z_d, aT) in enumerate([(z1_d, anT), (z2_d, bnT)]):
                    zT = szt.tile([128, N], BF16, tag="zT")
                    # transpose z into [d, i] layout via PE
                    for i in range(N // 128):
                        nat = sw.tile([128, 128], BF16, tag="nat")
                        nc.sync.dma_start(nat[:], z_d[i * 128 : (i + 1) * 128, :])
                        tps = sp1.tile([128, 128], BF16, tag="tps")
                        nc.tensor.transpose(tps[:], nat[:], ident[:])
                        nc.vector.tensor_copy(zT[:, i * 128 : (i + 1) * 128], tps[:])
                    # project + normalize, 512-wide chunks
                    for k in range(N // 512):
                        sl = slice(k * 512, (k + 1) * 512)
                        psA = sp2.tile([128, 512], F32, tag="psA")
                        nc.tensor.matmul(psA[:], w1sb[:], zT[:, sl])
                        expu = sw.tile([128, 512], F32, tag="expu")
                        nc.scalar.activation(expu[:], psA[:], AF.Exp, bias=b1sb[:])
                        relu = sw.tile([128, 512], F32, tag="relu")
                        nc.scalar.activation(relu[:], psA[:], AF.Relu, bias=b1sb[:])
                        # elu(y)+1 = min(exp(y),1) + max(y,0)
                        p1c = sw.tile([128, 512], BF16, tag="p1c")
                        nc.vector.scalar_tensor_tensor(
                            p1c[:], expu[:], 1.0, relu[:], OP.min, OP.add
                        )
                        psB = sp2.tile([128, 512], F32, tag="psB")
                        nc.tensor.matmul(psB[:], w2sb[:], p1c[:])
                        hc = sw.tile([128, 512], BF16, tag="hc")
                        nc.vector.tensor_scalar(hc[:], psB[:], b2psb[:], None, OP.add)
                        sq = sw.tile([128, 512], BF16, tag="sq")
                        nc.vector.tensor_mul(sq[:], hc[:], hc[:])
                        psC = sp1.tile([1, 512], F32, tag="psC")
                        nc.tensor.matmul(psC[:], ones_col_bf[:], sq[:])
                        lnq = sw.tile([1, 512], F32, tag="lnq")
                        nc.scalar.activation(lnq[:], psC[:], AF.Ln)
                        psD = sp2.tile([128, 512], F32, tag="psD")
                        nc.tensor.matmul(psD[:], ones_row_f[:], lnq[:])
                        invnb = sw.tile([128, 512], F32, tag="invnb")
                        nc.scalar.activation(invnb[:], psD[:], AF.Exp, scale=-0.5)
                        nc.vector.tensor_mul(aT[:, sl], invnb[:], hc[:])

                # num_i = exp(an_i . bn_i / tau) for local rows (cols 0..R)
                for q in range(R // 512):
                    sl = slice(q * 512, (q + 1) * 512)
                    prod = sw.tile([128, 512], F32, tag="prod")
                    nc.vector.tensor_mul(prod[:], anT[:, sl], bnT[:, sl])
                    psN = sp1.tile([1, 512], F32, tag="psC")
                    nc.tensor.matmul(psN[:], ones_col_f[:], prod[:])
                    numt = sw.tile([1, 512], F32, tag="numt")
                    nc.scalar.activation(numt[:], psN[:], AF.Exp, scale=INV_TAU)
                    nc.sync.dma_start(num_d[0:1, sl], numt[:])

            # ---------------- main loop: 3 similarity row-blocks ------------
            with tc.tile_pool(name="mp", bufs=1) as mp:
                colacc = mp.tile([128, N], F32, tag="colacc")
                nc.gpsimd.memset(colacc[:], 0.0)
                with (
                    tc.tile_pool(name="me", bufs=4) as me,
                    tc.tile_pool(name="ma", bufs=4) as ma,
                    tc.tile_pool(name="mpp", bufs=2, space="PSUM") as mpp,
                ):
                    mats = [(anT, anT, False), (anT, bnT, True), (bnT, bnT, False)]
                    for mi, (lhs, rhs, need_col) in enumerate(mats):
                        for m in range(R // 128):
                            lT = lhs[:, m * 128 : (m + 1) * 128]
                            acc8 = ma.tile([128, 8], F32, tag="acc8")
                            for jt in range(8):
                                ps = mpp.tile([128, 2048], F32, tag="mm")
                                for q in range(4):
                                    nc.tensor.matmul(
                                        ps[:, q * 512 : (q + 1) * 512],
                                        lT,
                                        rhs[:, jt * 2048 + q * 512 : jt * 2048 + (q + 1) * 512],
                                    )
                                E = me.tile([128, 2048], BF16, tag="E")
                                nc.scalar.activation(
                                    E[:],
                                    ps[:],
                                    AF.Exp,
                                    scale=INV_TAU,
                                    accum_out=acc8[:, jt : jt + 1],
                                )
                                if need_col:
                                    csl = slice(jt * 2048, (jt + 1) * 2048)
                                    nc.vector.scalar_tensor_tensor(
                                        colacc[:, csl], E[:], 1.0, colacc[:, csl],
                                        OP.mult, OP.add,
                                    )
                            nc.vector.tensor_reduce(
                                rs_sb[mi][:, m : m + 1], acc8[:],
                                mybir.AxisListType.X, OP.add,
                            )
                        nc.sync.dma_start(rs_d[mi][:], rs_sb[mi][:])

                # cs12[j] = sum over this core's rows of exp(S12)[.,j]
                with (
                    tc.tile_pool(name="cw", bufs=2) as cw,
                    tc.tile_pool(name="cpp", bufs=2, space="PSUM") as cpp,
                ):
                    for k in range(N // 512):
                        sl = slice(k * 512, (k + 1) * 512)
                        psK = cpp.tile([1, 512], F32, tag="psK")
                        nc.tensor.matmul(psK[:], ones_col_f[:], colacc[:, sl])
                        cst = cw.tile([1, 512], F32, tag="cst")
                        nc.vector.tensor_copy(cst[:], psK[:])
                        nc.sync.dma_start(cs12_d[0:1, sl], cst[:])

    return nc


def _get_nc():
    global _NC_CACHE
    if _NC_CACHE is None:
        _NC_CACHE = _build()
    return _NC_CACHE


def kernel(z1, z2, W1, b1, W2, b2):
    global LAST_RES
    bf = ml_dtypes.bfloat16
    z1 = np.asarray(z1, dtype=np.float32)
    z2 = np.asarray(z2, dtype=np.float32)
    W1 = np.asarray(W1, dtype=np.float32)
    W2 = np.asarray(W2, dtype=np.float32)
    b1 = np.asarray(b1, dtype=np.float32)
    b2 = np.asarray(b2, dtype=np.float32)
    # fold the "-1" of elu(y) = (min(exp y,1)+max(y,0)) - 1 into the 2nd bias
    b2p = (b2.astype(np.float64) - W2.astype(np.float64).sum(0)).astype(np.float32)

    nc = _get_nc()
    in_maps = []
    for c in range(NCORES):
        in_maps.append(
            {
                "z1": np.roll(z1, -c * R, axis=0).astype(bf),
                "z2": np.roll(z2, -c * R, axis=0).astype(bf),
                "w1": W1.astype(bf),
                "w2": W2.astype(bf),
                "b1": b1.reshape(D, 1).copy(),
                "b2p": b2p.reshape(D, 1).copy(),
            }
        )
    res = run_bass_kernel_spmd(nc, in_maps, list(range(NCORES)), **RUN_KWARGS)
    LAST_RES = res

    e2 = np.exp(np.float64(INV_TAU))
    rs11 = np.empty(N, np.float64)
    rs12 = np.empty(N, np.float64)
    rs22 = np.empty(N, np.float64)
    num = np.empty(N, np.float64)
    cs12 = np.zeros(N, np.float64)
    for c in range(NCORES):
        r = res.results[c]
        sl = slice(c * R, (c + 1) * R)
        rs11[sl] = r["rs0"].astype(np.float64).T.reshape(R)
        rs12[sl] = r["rs1"].astype(np.float64).T.reshape(R)
        rs22[sl] = r["rs2"].astype(np.float64).T.reshape(R)
        num[sl] = r["num"].astype(np.float64).reshape(R)
        cs12 += np.roll(r["cs12"].astype(np.float64).reshape(N), c * R)

    den1 = rs11 + rs12 - e2
    den2 = rs22 + cs12 - e2
    l1 = np.log(den1) - np.log(num)
    l2 = np.log(den2) - np.log(num)
    loss = np.mean(0.5 * (l1 + l2))
    return np.array(loss, dtype=np.float32)



# revision 13
# speedup vs baseline: 1.3095x; 1.3095x over previous
"""Trainium2 Bass kernel for the HGCA contrastive loss (nn_HGCA_10857677324785).

loss = mean over i of 0.5*(l1_i + l2_i) where
  h1 = elu(z1@W1+b1)@W2+b2 ; h2 likewise ; an, bn = l2-normalized rows
  l1_i = -log( exp(an_i.bn_i/tau) / (sum_j exp(an_i.an_j/tau)
               + sum_j exp(an_i.bn_j/tau) - e^{1/tau}) )
  l2_i symmetric with row sums of exp(bn@bn.T) and exp(bn@an.T).

Distribution: rows sharded over 8 cores (host rolls z1/z2 per core so the
core's rows sit at local rows [0,2048)). Each core computes the full
normalized projections, then:
  - S12 = an@bn.T row-block fully (rowsums via ACT accum, colsums via bf16
    column accumulators on DVE + PE ones-reduce)  [for l1 and l2]
  - S11/S22 exploit symmetry: each 128-row strip computes only the cyclic
    diagonal band d in [0,64] (d = col_block - row_block). Row sums at
    distance >= 65 are recovered from column sums of the transposed blocks
    computed by the mirror strips (on whatever core owns them). This saves
    ~1/3 of the exp work, the hard floor of this kernel (exp runs only on
    the scalar engine at 128 lanes/cycle).
Host assembles the scalar loss from O(N) partial sums in float64.
"""

import ml_dtypes
import numpy as np

import concourse.bass as bass
import concourse.tile as tile
from concourse import mybir
from concourse.bass_utils import run_bass_kernel_spmd

N = 16384
D = 128
NCORES = 8
R = N // NCORES          # 2048 rows per core
NSTRIP = R // 128        # 16 strips of 128 rows
NPANEL = N // 2048       # 8 col panels for S12
ND = 4                   # d-panels (d in [1,16],[17,32],[33,48],[49,64])
CACC_W = 79 * 128        # colacc11/22 width: col blocks 1..79
INV_TAU = 2.0
F32 = mybir.dt.float32
BF16 = mybir.dt.bfloat16
AF = mybir.ActivationFunctionType
OP = mybir.AluOpType

# This walrus build supports at most 2 sync waits per instruction; Tile's sem
# assignment freely emits 3-11. Post-pass: hoist excess waits onto injected
# same-engine EventSemaphore fillers (engine queues are FIFO, so waits on an
# earlier filler happen-before the original instruction executes).

_MAX_WAITS = 1


def _split_waits(nc):
    for fn in nc.m.functions:
        for bb in fn.blocks:
            insts = list(bb.instructions)
            out = []
            changed = False
            for inst in insts:
                si = inst.sync_info
                w = list(si.on_wait) if si and si.on_wait else []
                if len(w) > _MAX_WAITS:
                    changed = True
                    extra, keep = w[:-_MAX_WAITS], w[-_MAX_WAITS:]
                    for i in range(0, len(extra), _MAX_WAITS):
                        f = mybir.InstEventSemaphore(
                            name=f"{inst.name}_wsplit{i}",
                            engine=inst.engine,
                            ins=[],
                            outs=[],
                            sync_info=mybir.SyncInfo(
                                on_wait=extra[i : i + _MAX_WAITS], on_update=[]
                            ),
                        )
                        out.append(f)
                    inst.sync_info = mybir.SyncInfo(
                        on_wait=keep,
                        on_update=list(si.on_update) if si.on_update else [],
                    )
                out.append(inst)
            if changed:
                bb.instructions = out


from concourse.vector_clock import ScopedClock


def _patched_drain_and_barrier(self, tick_clock, wait_clock):
    nc = self.nc
    drain_inst = nc.sync.drain()
    wait_clock.add_sem_waits(
        drain_inst.ins, ScopedClock({None: tick_clock.global_clock})
    )
    nc.all_engine_barrier()
    assert self.sems is not None
    popped = nc._tile_sem_poison_stack.pop()
    assert popped is self._sem_poison
    nc.clear_and_free_semaphores(list(self.sems.allocated().values()))
    nc.all_engine_barrier()
    _split_waits(nc)


tile.TileContext._drain_and_barrier = _patched_drain_and_barrier

_NC_CACHE = None
RUN_KWARGS: dict = {}
LAST_RES = None


def _build():
    nc = bass.Bass("TRN2", target_bir_lowering=False, debug=False)

    z1_d = nc.dram_tensor("z1", [N, D], BF16, kind="ExternalInput").ap()
    z2_d = nc.dram_tensor("z2", [N, D], BF16, kind="ExternalInput").ap()
    w1_d = nc.dram_tensor("w1", [D, D], BF16, kind="ExternalInput").ap()
    w2_d = nc.dram_tensor("w2", [D, D], BF16, kind="ExternalInput").ap()
    b1_d = nc.dram_tensor("b1", [D, 1], F32, kind="ExternalInput").ap()
    b2p_d = nc.dram_tensor("b2p", [D, 1], F32, kind="ExternalInput").ap()

    rs_d = [
        nc.dram_tensor(f"rs{i}", [128, NSTRIP], F32, kind="ExternalOutput").ap()
        for i in range(3)
    ]  # 0: S11 own-band rowsums, 1: S12 rowsums, 2: S22 own-band rowsums
    cs12_d = nc.dram_tensor("cs12", [1, N], BF16, kind="ExternalOutput").ap()
    cs11_d = nc.dram_tensor("cs11", [1, CACC_W], BF16, kind="ExternalOutput").ap()
    cs22_d = nc.dram_tensor("cs22", [1, CACC_W], BF16, kind="ExternalOutput").ap()
    num_d = nc.dram_tensor("numd", [1, R], BF16, kind="ExternalOutput").ap()
    scrP_d = nc.dram_tensor("scrP", [16, 2048], F32, kind="Internal").ap()
    scrU_d = nc.dram_tensor("scrU", [16, 2048], BF16, kind="Internal").ap()

    with tile.TileContext(nc) as tc:
        with (
            tc.tile_pool(name="pers", bufs=1) as pers,
            tc.tile_pool(name="cacc", bufs=1) as caccp,
            tc.tile_pool(name="cacc12", bufs=2) as cacc12p,
            tc.tile_pool(name="consts", bufs=1) as consts,
            tc.tile_pool(name="zt", bufs=1) as ztp,
            tc.tile_pool(name="proj", bufs=2) as projp,
            tc.tile_pool(name="npk", bufs=1) as npkp,
            tc.tile_pool(name="nrows", bufs=2) as nrowp,
            tc.tile_pool(name="irows", bufs=3) as irowp,
            tc.tile_pool(name="epool", bufs=3) as epool,
            tc.tile_pool(name="cs", bufs=2) as csp,
            tc.tile_pool(name="pA", bufs=1, space="PSUM") as pA,
            tc.tile_pool(name="pB", bufs=1, space="PSUM") as pB,
        ):
            # ---------------- persistent / const tiles ----------------
            anT = pers.tile([128, N], BF16, tag="anT")
            bnT = pers.tile([128, N], BF16, tag="bnT")
            acc11 = pers.tile([128, NSTRIP * 5], F32, tag="acc11")
            acc22 = pers.tile([128, NSTRIP * 5], F32, tag="acc22")
            acc12 = pers.tile([128, NSTRIP * 8], F32, tag="acc12")
            rs_sb = [
                pers.tile([128, NSTRIP], F32, tag=f"rs{i}", name=f"rs_sb{i}")
                for i in range(3)
            ]
            num_sb = pers.tile([1, R], BF16, tag="numsb")
            # packed per-column squared norms / inverse norms:
            # nsqP[p, t*128 + 16k + q] = |h_t column (k*2048 + q*128 + p)|^2
            nsqP = npkp.tile([128, 256], F32, tag="nsqP")
            invP = npkp.tile([128, 256], BF16, tag="invP")

            ones_col_bf = consts.tile([128, 1], BF16, tag="ocb")
            nc.gpsimd.memset(ones_col_bf[:], 1.0)
            ones_row_bf = consts.tile([1, 128], BF16, tag="orb")
            nc.gpsimd.memset(ones_row_bf[:], 1.0)
            w1sb = consts.tile([128, 128], BF16, tag="w1")
            nc.sync.dma_start(w1sb[:], w1_d[:])
            w2sb = consts.tile([128, 128], BF16, tag="w2")
            nc.sync.dma_start(w2sb[:], w2_d[:])
            b1sb = consts.tile([128, 1], F32, tag="b1")
            nc.sync.dma_start(b1sb[:], b1_d[:])
            b2psb = consts.tile([128, 1], F32, tag="b2p")
            nc.sync.dma_start(b2psb[:], b2p_d[:])

            # PSUM [128,2048] alternation between the two 4-bank pools
            _ps_state = [0]

            def ps_tile():
                _ps_state[0] ^= 1
                pool = pA if _ps_state[0] else pB
                return pool.tile([128, 2048], F32, tag="mm", name="ps")

            def mm_fill(ps, lhsT, rhs_base, cols, ps_off=0):
                q = 0
                while q < cols:
                    w = min(512, cols - q)
                    nc.tensor.matmul(
                        ps[:, ps_off + q : ps_off + q + w],
                        lhsT,
                        rhs_base[:, q : q + w],
                    )
                    q += w

            # ---------------- projection ----------------
            def transpose_in(z_d):
                zT = ztp.tile([128, N], BF16, tag="zT", name="zT")
                for q in range(4):
                    nc.sync.dma_start_transpose(
                        zT[:, q * 4096 : (q + 1) * 4096],
                        z_d[q * 4096 : (q + 1) * 4096, :],
                    )
                return zT

            def pass1_chunk(zT, hT, k):
                """hT[:, chunk k] <- elu(zT @ W1 + b1) + 1"""
                sl = slice(k * 2048, (k + 1) * 2048)
                psA = ps_tile()
                mm_fill(psA, w1sb[:], zT[:, sl], 2048)
                e1 = projp.tile([128, 2048], BF16, tag="e1")
                nc.scalar.activation(e1[:], psA[:], AF.Exp, bias=b1sb[:])
                r1 = projp.tile([128, 2048], BF16, tag="r1")
                nc.vector.tensor_scalar(r1[:], psA[:], b1sb[:], 0.0, OP.add, OP.max)
                nc.vector.scalar_tensor_tensor(
                    hT[:, sl], e1[:], 1.0, r1[:], OP.min, OP.add
                )

            invrows = {}

            def pass2_chunk(hT, t, k):
                """hT chunk k <- elu1 @ W2 + b2'; per-column inverse norms
                computed via a DRAM-bounced pack -> [128,16] ln/exp ->
                DRAM-bounced unpack into invrows[(t, k)]."""
                sl = slice(k * 2048, (k + 1) * 2048)
                psB = ps_tile()
                mm_fill(psB, w2sb[:], hT[:, sl], 2048)
                nc.vector.tensor_scalar(hT[:, sl], psB[:], b2psb[:], None, OP.add)
                sq = projp.tile([128, 2048], BF16, tag="sq")
                nc.vector.tensor_tensor(sq[:], hT[:, sl], hT[:, sl], OP.mult)
                psN = ps_tile()
                mm_fill(psN[0:1, :], ones_col_bf[:], sq[:], 2048)
                nrow = nrowp.tile([1, 2048], F32, tag="nrow")
                nc.vector.tensor_copy(nrow[:], psN[0:1, :])
                srow = t * 8 + k
                nc.sync.dma_start(scrP_d[srow : srow + 1, :], nrow[:])
                csl = slice(t * 128 + 16 * k, t * 128 + 16 * k + 16)
                nc.sync.dma_start(
                    nsqP[:, csl],
                    scrP_d[srow : srow + 1, :].rearrange("a (q p) -> (a p) q", p=128),
                )
                nc.scalar.activation(nsqP[:, csl], nsqP[:, csl], AF.Ln)
                nc.scalar.activation(invP[:, csl], nsqP[:, csl], AF.Exp, scale=-0.5)
                nc.sync.dma_start(
                    scrU_d[srow : srow + 1, :].rearrange("a (q p) -> (a p) q", p=128),
                    invP[:, csl],
                )
                invrow = irowp.tile([1, 2048], BF16, tag="invrow")
                nc.sync.dma_start(invrow[:], scrU_d[srow : srow + 1, :])
                invrows[(t, k)] = invrow

            def apply_chunk(hT, t, k):
                sl = slice(k * 2048, (k + 1) * 2048)
                invrow = invrows.pop((t, k))
                psI = ps_tile()
                mm_fill(psI, ones_row_bf[:], invrow[:], 2048)
                nc.vector.tensor_tensor(hT[:, sl], hT[:, sl], psI[:], OP.mult)

            # ------------- symmetric matrices: cyclic diagonal band -------------
            def csum_out(cacc_sl, cs_dram_sl, width):
                psC = ps_tile()
                mm_fill(psC[0:1, :], ones_col_bf[:], cacc_sl, width)
                cs_sb = csp.tile([1, 2048], BF16, tag="cssb")
                nc.vector.tensor_copy(cs_sb[0:1, 0:width], psC[0:1, 0:width])
                nc.sync.dma_start(cs_dram_sl, cs_sb[0:1, 0:width])

            def sym_band_units(xT, acc, cs_dram):
                colacc = caccp.tile([128, CACC_W], BF16, tag="cacc", name="colacc")
                units = []

                def u_init():
                    nc.gpsimd.memset(colacc[:], 0.0)
                    psD = ps_tile()
                    for rl in range(NSTRIP):
                        nc.tensor.matmul(
                            psD[:, rl * 128 : (rl + 1) * 128],
                            xT[:, rl * 128 : (rl + 1) * 128],
                            xT[:, rl * 128 : (rl + 1) * 128],
                        )
                    eD = epool.tile([128, 2048], BF16, tag="E", name="eD")
                    nc.scalar.activation(eD[:], psD[:], AF.Exp, scale=INV_TAU)
                    for rl in range(NSTRIP):
                        nc.vector.tensor_reduce(
                            acc[:, rl * 5 + 4 : rl * 5 + 5],
                            eD[:, rl * 128 : (rl + 1) * 128],
                            mybir.AxisListType.X,
                            OP.add,
                        )

                units.append(u_init)

                def u_strip(dp, rl):
                    lhsT = xT[:, rl * 128 : (rl + 1) * 128]
                    c0 = (rl + 1 + 16 * dp) * 128
                    ps = ps_tile()
                    mm_fill(ps, lhsT, xT[:, c0 : c0 + 2048], 2048)
                    E = epool.tile([128, 2048], BF16, tag="E", name="E")
                    nc.scalar.activation(
                        E[:],
                        ps[:],
                        AF.Exp,
                        scale=INV_TAU,
                        accum_out=acc[:, rl * 5 + dp : rl * 5 + dp + 1],
                    )
                    # colacc excludes d=64 (the last block of dp==3)
                    w = 1920 if dp == ND - 1 else 2048
                    t0 = (rl + 16 * dp) * 128
                    nc.vector.tensor_tensor(
                        colacc[:, t0 : t0 + w],
                        E[:, 0:w],
                        colacc[:, t0 : t0 + w],
                        OP.add,
                    )

                for dp in range(ND):
                    for rl in range(NSTRIP):
                        units.append(lambda dp=dp, rl=rl: u_strip(dp, rl))

                def u_csum(piece):
                    t0 = piece * 2048
                    w = min(2048, CACC_W - t0)
                    csum_out(colacc[:, t0 : t0 + w], cs_dram[0:1, t0 : t0 + w], w)

                for piece in range(5):
                    units.append(lambda piece=piece: u_csum(piece))
                return units

            # ---- schedule ----
            s11_units = None
            s11_pos = [0]

            def drain_s11(n):
                for _ in range(n):
                    if s11_pos[0] < len(s11_units):
                        s11_units[s11_pos[0]]()
                        s11_pos[0] += 1

            zT1 = transpose_in(z1_d)
            for k in range(8):
                pass1_chunk(zT1, anT, k)
            zT2 = transpose_in(z2_d)  # DMA overlaps z1 PASS2 (zT1 dead)

            for k in range(8):
                pass2_chunk(anT, 0, k)
                if k >= 2:
                    apply_chunk(anT, 0, k - 2)
                if k == 3:
                    s11_units = sym_band_units(anT, acc11, cs11_d)
                if k >= 4:
                    drain_s11(2)
            apply_chunk(anT, 0, 6)
            apply_chunk(anT, 0, 7)
            drain_s11(2)

            for k in range(8):
                pass1_chunk(zT2, bnT, k)
                drain_s11(2)
            for k in range(8):
                pass2_chunk(bnT, 1, k)
                if k >= 2:
                    apply_chunk(bnT, 1, k - 2)
                drain_s11(2)
            apply_chunk(bnT, 1, 6)
            apply_chunk(bnT, 1, 7)

            # num: diagonal dots an_i . bn_i for local rows [0, 2048)
            prod = projp.tile([128, 2048], BF16, tag="e1", name="prod")
            nc.vector.tensor_tensor(prod[:], anT[:, 0:R], bnT[:, 0:R], OP.mult)
            psNm = ps_tile()
            mm_fill(psNm[0:1, :], ones_col_bf[:], prod[:], 2048)
            nc.vector.tensor_copy(num_sb[:], psNm[0:1, :])
            nc.sync.dma_start(num_d[:], num_sb[:])

            drain_s11(len(s11_units) - s11_pos[0] - 5)  # keep the 5 csums

            # ------------------------- S12 full row block -------------------------
            for p in range(NPANEL):
                sl = slice(p * 2048, (p + 1) * 2048)
                colacc12 = cacc12p.tile([128, 2048], BF16, tag="cacc12", name="colacc12")
                for rl in range(NSTRIP):
                    lhsT = anT[:, rl * 128 : (rl + 1) * 128]
                    ps = ps_tile()
                    mm_fill(ps, lhsT, bnT[:, sl], 2048)
                    E = epool.tile([128, 2048], BF16, tag="E", name="E")
                    nc.scalar.activation(
                        E[:],
                        ps[:],
                        AF.Exp,
                        scale=INV_TAU,
                        accum_out=acc12[:, rl * 8 + p : rl * 8 + p + 1],
                    )
                    if rl == 0:
                        nc.vector.tensor_copy(colacc12[:], E[:])
                    else:
                        nc.vector.tensor_tensor(colacc12[:], E[:], colacc12[:], OP.add)
                csum_out(colacc12[:], cs12_d[0:1, sl], 2048)
                drain_s11(1)  # the 5 deferred S11 csums ride the first panels

            for u in sym_band_units(bnT, acc22, cs22_d):
                u()

            # ------------------------- rowsum reduction -------------------------
            for rl in range(NSTRIP):
                nc.vector.tensor_reduce(
                    rs_sb[0][:, rl : rl + 1],
                    acc11[:, rl * 5 : (rl + 1) * 5],
                    mybir.AxisListType.X,
                    OP.add,
                )
                nc.vector.tensor_reduce(
                    rs_sb[1][:, rl : rl + 1],
                    acc12[:, rl * 8 : (rl + 1) * 8],
                    mybir.AxisListType.X,
                    OP.add,
                )
                nc.vector.tensor_reduce(
                    rs_sb[2][:, rl : rl + 1],
                    acc22[:, rl * 5 : (rl + 1) * 5],
                    mybir.AxisListType.X,
                    OP.add,
                )
            for i in range(3):
                nc.sync.dma_start(rs_d[i][:], rs_sb[i][:])

    return nc


def _get_nc():
    global _NC_CACHE
    if _NC_CACHE is None:
        _NC_CACHE = _build()
    return _NC_CACHE


def kernel(z1, z2, W1, b1, W2, b2):
    global LAST_RES
    bf = ml_dtypes.bfloat16
    z1 = np.asarray(z1, dtype=np.float32)
    z2 = np.asarray(z2, dtype=np.float32)
    W1 = np.asarray(W1, dtype=np.float32)
    W2 = np.asarray(W2, dtype=np.float32)
    b1 = np.asarray(b1, dtype=np.float32)
    b2 = np.asarray(b2, dtype=np.float32)
    # fold the "-1" of elu(y) = (min(exp y,1)+max(y,0)) - 1 into the 2nd bias
    b2p = (b2.astype(np.float64) - W2.astype(np.float64).sum(0)).astype(np.float32)

    nc = _get_nc()
    in_maps = []
    for c in range(NCORES):
        in_maps.append(
            {
                "z1": np.roll(z1, -c * R, axis=0).astype(bf),
                "z2": np.roll(z2, -c * R, axis=0).astype(bf),
                "w1": W1.astype(bf),
                "w2": W2.astype(bf),
                "b1": b1.reshape(D, 1).copy(),
                "b2p": b2p.reshape(D, 1).copy(),
            }
        )
    res = run_bass_kernel_spmd(nc, in_maps, list(range(NCORES)), **RUN_KWARGS)
    LAST_RES = res

    e2 = np.exp(np.float64(INV_TAU))
    rs11o = np.empty(N, np.float64)
    rs12o = np.empty(N, np.float64)
    rs22o = np.empty(N, np.float64)
    lognum = np.empty(N, np.float64)
    cs12 = np.zeros(N, np.float64)
    cs11 = np.zeros(N, np.float64)
    cs22 = np.zeros(N, np.float64)
    for c in range(NCORES):
        r = res.results[c]
        sl = slice(c * R, (c + 1) * R)
        rs11o[sl] = r["rs0"].astype(np.float64).T.reshape(R)
        rs12o[sl] = r["rs1"].astype(np.float64).T.reshape(R)
        rs22o[sl] = r["rs2"].astype(np.float64).T.reshape(R)
        lognum[sl] = r["numd"].astype(np.float64).reshape(R) * INV_TAU
        cs12 += np.roll(r["cs12"].astype(np.float64).reshape(N), c * R)
        e11 = np.zeros(N, np.float64)
        e11[128 : 128 + CACC_W] = r["cs11"].astype(np.float64).reshape(CACC_W)
        cs11 += np.roll(e11, c * R)
        e22 = np.zeros(N, np.float64)
        e22[128 : 128 + CACC_W] = r["cs22"].astype(np.float64).reshape(CACC_W)
        cs22 += np.roll(e22, c * R)

    den1 = rs11o + cs11 + rs12o - e2
    den2 = rs22o + cs22 + cs12 - e2
    loss = np.mean(0.5 * (np.log(den1) + np.log(den2)) - lognum)
    return np.array(loss, dtype=np.float32)


# revision 18
# speedup vs baseline: 1.5254x; 1.1649x over previous
"""Trainium2 Bass kernel for the HGCA contrastive loss (nn_HGCA_10857677324785).

loss = mean over i of 0.5*(l1_i + l2_i) where
  h1 = elu(z1@W1+b1)@W2+b2 ; h2 likewise ; an, bn = l2-normalized rows
  l1_i = -log( exp(an_i.bn_i/tau) / (sum_j exp(an_i.an_j/tau)
               + sum_j exp(an_i.bn_j/tau) - e^{1/tau}) )
  l2_i symmetric with row sums of exp(bn@bn.T) and exp(bn@an.T).

Distribution: rows sharded over 8 cores (host rolls z1/z2 per core so the
core's rows sit at local rows [0,2048)). Each core computes the full
normalized projections, then:
  - S12 = an@bn.T row-block fully (rowsums via ACT accum, colsums via bf16
    column accumulators on DVE + PE ones-reduce)  [for l1 and l2]
  - S11/S22 exploit symmetry: each 128-row strip computes only the cyclic
    diagonal band d in [0,64] (d = col_block - row_block). Row sums at
    distance >= 65 are recovered from column sums of the transposed blocks
    computed by the mirror strips (on whatever core owns them). This saves
    ~1/3 of the exp work, the hard floor of this kernel (exp runs only on
    the scalar engine at 128 lanes/cycle).
Host assembles the scalar loss from O(N) partial sums in float64.
"""

import ml_dtypes
import numpy as np

import concourse.bass as bass
import concourse.tile as tile
from concourse import mybir
from concourse.bass_utils import run_bass_kernel_spmd

N = 16384
D = 128
NCORES = 8
R = N // NCORES          # 2048 rows per core
NSTRIP = R // 128        # 16 strips of 128 rows
NPANEL = N // 2048       # 8 col panels for S12
ND = 4                   # d-panels (d in [1,16],[17,32],[33,48],[49,64])
CACC_W = 79 * 128        # colacc11/22 width: col blocks 1..79
INV_TAU = 2.0
F32 = mybir.dt.float32
BF16 = mybir.dt.bfloat16
AF = mybir.ActivationFunctionType
OP = mybir.AluOpType

# This walrus build supports at most 2 sync waits per instruction; Tile's sem
# assignment freely emits 3-11. Post-pass: hoist excess waits onto injected
# same-engine EventSemaphore fillers (engine queues are FIFO, so waits on an
# earlier filler happen-before the original instruction executes).

_MAX_WAITS = 1


def _split_waits(nc):
    for fn in nc.m.functions:
        for bb in fn.blocks:
            insts = list(bb.instructions)
            out = []
            changed = False
            for inst in insts:
                si = inst.sync_info
                w = list(si.on_wait) if si and si.on_wait else []
                if len(w) > _MAX_WAITS:
                    changed = True
                    extra, keep = w[:-_MAX_WAITS], w[-_MAX_WAITS:]
                    for i in range(0, len(extra), _MAX_WAITS):
                        f = mybir.InstEventSemaphore(
                            name=f"{inst.name}_wsplit{i}",
                            engine=inst.engine,
                            ins=[],
                            outs=[],
                            sync_info=mybir.SyncInfo(
                                on_wait=extra[i : i + _MAX_WAITS], on_update=[]
                            ),
                        )
                        out.append(f)
                    inst.sync_info = mybir.SyncInfo(
                        on_wait=keep,
                        on_update=list(si.on_update) if si.on_update else [],
                    )
                out.append(inst)
            if changed:
                bb.instructions = out


from concourse.vector_clock import ScopedClock


def _patched_drain_and_barrier(self, tick_clock, wait_clock):
    nc = self.nc
    drain_inst = nc.sync.drain()
    wait_clock.add_sem_waits(
        drain_inst.ins, ScopedClock({None: tick_clock.global_clock})
    )
    nc.all_engine_barrier()
    assert self.sems is not None
    popped = nc._tile_sem_poison_stack.pop()
    assert popped is self._sem_poison
    nc.clear_and_free_semaphores(list(self.sems.allocated().values()))
    nc.all_engine_barrier()
    _split_waits(nc)


tile.TileContext._drain_and_barrier = _patched_drain_and_barrier

_NC_CACHE = None
RUN_KWARGS: dict = {}
LAST_RES = None


def _build():
    nc = bass.Bass("TRN2", target_bir_lowering=False, debug=False)

    z1_d = nc.dram_tensor("z1", [N, D], BF16, kind="ExternalInput").ap()
    z2_d = nc.dram_tensor("z2", [N, D], BF16, kind="ExternalInput").ap()
    w1_d = nc.dram_tensor("w1", [D, D], BF16, kind="ExternalInput").ap()
    w2_d = nc.dram_tensor("w2", [D, D], BF16, kind="ExternalInput").ap()
    b1_d = nc.dram_tensor("b1", [D, 1], F32, kind="ExternalInput").ap()
    b2p_d = nc.dram_tensor("b2p", [D, 1], F32, kind="ExternalInput").ap()

    rs_d = [
        nc.dram_tensor(f"rs{i}", [128, NSTRIP], F32, kind="ExternalOutput").ap()
        for i in range(3)
    ]  # 0: S11 own-band rowsums, 1: S12 rowsums, 2: S22 own-band rowsums
    cs12_d = nc.dram_tensor("cs12", [128, N], BF16, kind="ExternalOutput").ap()
    cs11_d = nc.dram_tensor("cs11", [128, CACC_W], BF16, kind="ExternalOutput").ap()
    cs22_d = nc.dram_tensor("cs22", [128, CACC_W], BF16, kind="ExternalOutput").ap()
    num_d = nc.dram_tensor("numd", [1, R], BF16, kind="ExternalOutput").ap()
    scrP_d = nc.dram_tensor("scrP", [16, 2048], F32, kind="Internal").ap()
    scrU_d = nc.dram_tensor("scrU", [16, 2048], BF16, kind="Internal").ap()

    with tile.TileContext(nc) as tc:
        with (
            tc.tile_pool(name="pers", bufs=1) as pers,
            tc.tile_pool(name="cacc", bufs=1) as caccp,
            tc.tile_pool(name="cacc12", bufs=2) as cacc12p,
            tc.tile_pool(name="consts", bufs=1) as consts,
            tc.tile_pool(name="zt", bufs=1) as ztp,
            tc.tile_pool(name="proj", bufs=2) as projp,
            tc.tile_pool(name="npk", bufs=1) as npkp,
            tc.tile_pool(name="nrows", bufs=2) as nrowp,
            tc.tile_pool(name="irows", bufs=3) as irowp,
            tc.tile_pool(name="epool", bufs=3) as epool,
            tc.tile_pool(name="pA", bufs=1, space="PSUM") as pA,
            tc.tile_pool(name="pB", bufs=1, space="PSUM") as pB,
        ):
            # ---------------- persistent / const tiles ----------------
            anT = pers.tile([128, N], BF16, tag="anT")
            bnT = pers.tile([128, N], BF16, tag="bnT")
            acc11 = pers.tile([128, NSTRIP * 5], F32, tag="acc11")
            acc22 = pers.tile([128, NSTRIP * 5], F32, tag="acc22")
            acc12 = pers.tile([128, NSTRIP * 8], F32, tag="acc12")
            rs_sb = [
                pers.tile([128, NSTRIP], F32, tag=f"rs{i}", name=f"rs_sb{i}")
                for i in range(3)
            ]
            num_sb = pers.tile([1, R], BF16, tag="numsb")
            # packed per-column squared norms / inverse norms:
            # nsqP[p, t*128 + 16k + q] = |h_t column (k*2048 + q*128 + p)|^2
            nsqP = npkp.tile([128, 256], F32, tag="nsqP")
            invP = npkp.tile([128, 256], BF16, tag="invP")

            ones_col_bf = consts.tile([128, 1], BF16, tag="ocb")
            nc.gpsimd.memset(ones_col_bf[:], 1.0)
            ones_row_bf = consts.tile([1, 128], BF16, tag="orb")
            nc.gpsimd.memset(ones_row_bf[:], 1.0)
            w1sb = consts.tile([128, 128], BF16, tag="w1")
            nc.sync.dma_start(w1sb[:], w1_d[:])
            w2sb = consts.tile([128, 128], BF16, tag="w2")
            nc.sync.dma_start(w2sb[:], w2_d[:])
            b1sb = consts.tile([128, 1], F32, tag="b1")
            nc.sync.dma_start(b1sb[:], b1_d[:])
            b2psb = consts.tile([128, 1], F32, tag="b2p")
            nc.sync.dma_start(b2psb[:], b2p_d[:])

            # PSUM [128,2048] alternation between the two 4-bank pools
            _ps_state = [0]

            def ps_tile():
                _ps_state[0] ^= 1
                pool = pA if _ps_state[0] else pB
                return pool.tile([128, 2048], F32, tag="mm", name="ps")

            def mm_fill(ps, lhsT, rhs_base, cols, ps_off=0):
                q = 0
                while q < cols:
                    w = min(512, cols - q)
                    nc.tensor.matmul(
                        ps[:, ps_off + q : ps_off + q + w],
                        lhsT,
                        rhs_base[:, q : q + w],
                    )
                    q += w

            # ---------------- projection ----------------
            def transpose_in(z_d):
                zT = ztp.tile([128, N], BF16, tag="zT", name="zT")
                for q in range(4):
                    nc.sync.dma_start_transpose(
                        zT[:, q * 4096 : (q + 1) * 4096],
                        z_d[q * 4096 : (q + 1) * 4096, :],
                    )
                return zT

            def pass1_chunk(zT, hT, k, relu_act=False):
                """hT[:, chunk k] <- elu(zT @ W1 + b1) + 1"""
                sl = slice(k * 2048, (k + 1) * 2048)
                psA = ps_tile()
                mm_fill(psA, w1sb[:], zT[:, sl], 2048)
                e1 = projp.tile([128, 2048], BF16, tag="e1")
                nc.scalar.activation(e1[:], psA[:], AF.Exp, bias=b1sb[:])
                r1 = projp.tile([128, 2048], BF16, tag="r1")
                if relu_act:
                    nc.scalar.activation(r1[:], psA[:], AF.Relu, bias=b1sb[:])
                else:
                    nc.vector.tensor_scalar(r1[:], psA[:], b1sb[:], 0.0, OP.add, OP.max)
                nc.vector.scalar_tensor_tensor(
                    hT[:, sl], e1[:], 1.0, r1[:], OP.min, OP.add
                )

            invrows = {}

            def pass2_chunk(hT, t, k):
                """hT chunk k <- elu1 @ W2 + b2'; per-column inverse norms
                computed via a DRAM-bounced pack -> [128,16] ln/exp ->
                DRAM-bounced unpack into invrows[(t, k)]."""
                sl = slice(k * 2048, (k + 1) * 2048)
                psB = ps_tile()
                mm_fill(psB, w2sb[:], hT[:, sl], 2048)
                nc.scalar.activation(hT[:, sl], psB[:], AF.Identity, bias=b2psb[:])
                sq = projp.tile([128, 2048], BF16, tag="sq")
                nc.vector.tensor_tensor(sq[:], hT[:, sl], hT[:, sl], OP.mult)
                psN = ps_tile()
                mm_fill(psN[0:1, :], ones_col_bf[:], sq[:], 2048)
                nrow = nrowp.tile([1, 2048], F32, tag="nrow")
                nc.scalar.activation(nrow[:], psN[0:1, :], AF.Copy)
                srow = t * 8 + k
                nc.sync.dma_start(scrP_d[srow : srow + 1, :], nrow[:])
                csl = slice(t * 128 + 16 * k, t * 128 + 16 * k + 16)
                nc.sync.dma_start(
                    nsqP[:, csl],
                    scrP_d[srow : srow + 1, :].rearrange("a (q p) -> (a p) q", p=128),
                )
                nc.scalar.activation(nsqP[:, csl], nsqP[:, csl], AF.Ln)
                nc.scalar.activation(invP[:, csl], nsqP[:, csl], AF.Exp, scale=-0.5)
                nc.sync.dma_start(
                    scrU_d[srow : srow + 1, :].rearrange("a (q p) -> (a p) q", p=128),
                    invP[:, csl],
                )
                invnb = irowp.tile([128, 2048], BF16, tag="invnb")
                nc.sync.dma_start(
                    invnb[:], scrU_d[srow : srow + 1, :].broadcast_to((128, 2048))
                )
                invrows[(t, k)] = invnb

            def apply_chunk(hT, t, k):
                sl = slice(k * 2048, (k + 1) * 2048)
                invnb = invrows.pop((t, k))
                nc.vector.tensor_tensor(hT[:, sl], hT[:, sl], invnb[:], OP.mult)

            # ------------- symmetric matrices: cyclic diagonal band -------------
            def sym_band_units(xT, acc, cs_dram):
                colacc = caccp.tile([128, CACC_W], BF16, tag="cacc", name="colacc")
                units = []

                def u_init():
                    nc.gpsimd.memset(colacc[:], 0.0)
                    psD = ps_tile()
                    for rl in range(NSTRIP):
                        nc.tensor.matmul(
                            psD[:, rl * 128 : (rl + 1) * 128],
                            xT[:, rl * 128 : (rl + 1) * 128],
                            xT[:, rl * 128 : (rl + 1) * 128],
                        )
                    eD = epool.tile([128, 2048], BF16, tag="E", name="eD")
                    nc.scalar.activation(eD[:], psD[:], AF.Exp, scale=INV_TAU)
                    for rl in range(NSTRIP):
                        nc.vector.tensor_reduce(
                            acc[:, rl * 5 + 4 : rl * 5 + 5],
                            eD[:, rl * 128 : (rl + 1) * 128],
                            mybir.AxisListType.X,
                            OP.add,
                        )

                units.append(u_init)

                def u_strip(dp, rl):
                    lhsT = xT[:, rl * 128 : (rl + 1) * 128]
                    c0 = (rl + 1 + 16 * dp) * 128
                    ps = ps_tile()
                    mm_fill(ps, lhsT, xT[:, c0 : c0 + 2048], 2048)
                    E = epool.tile([128, 2048], BF16, tag="E", name="E")
                    nc.scalar.activation(
                        E[:],
                        ps[:],
                        AF.Exp,
                        scale=INV_TAU,
                        accum_out=acc[:, rl * 5 + dp : rl * 5 + dp + 1],
                    )
                    # colacc excludes d=64 (the last block of dp==3)
                    w = 1920 if dp == ND - 1 else 2048
                    t0 = (rl + 16 * dp) * 128
                    nc.vector.tensor_tensor(
                        colacc[:, t0 : t0 + w],
                        E[:, 0:w],
                        colacc[:, t0 : t0 + w],
                        OP.add,
                    )

                for dp in range(ND):
                    for rl in range(NSTRIP):
                        units.append(lambda dp=dp, rl=rl: u_strip(dp, rl))

                def u_out():
                    nc.sync.dma_start(cs_dram[:, :], colacc[:])

                units.append(u_out)
                return units

            # ---- schedule ----
            s11_units = None
            s11_pos = [0]

            def drain_s11(n):
                for _ in range(n):
                    if s11_pos[0] < len(s11_units):
                        s11_units[s11_pos[0]]()
                        s11_pos[0] += 1

            zT1 = transpose_in(z1_d)
            for k in range(8):
                pass1_chunk(zT1, anT, k, relu_act=True)
            zT2 = transpose_in(z2_d)  # DMA overlaps z1 PASS2 (zT1 dead)

            # fused: z1 PASS2+normalize with z2 PASS1 (independent tensors)
            for k in range(8):
                pass2_chunk(anT, 0, k)
                pass1_chunk(zT2, bnT, k)
                if k >= 2:
                    apply_chunk(anT, 0, k - 2)
                if k == 3:
                    s11_units = sym_band_units(anT, acc11, cs11_d)
                if k >= 4:
                    drain_s11(2)
            apply_chunk(anT, 0, 6)
            apply_chunk(anT, 0, 7)
            drain_s11(2)

            for k in range(8):
                pass2_chunk(bnT, 1, k)
                if k >= 2:
                    apply_chunk(bnT, 1, k - 2)
                drain_s11(3)
            apply_chunk(bnT, 1, 6)
            apply_chunk(bnT, 1, 7)

            # num: diagonal dots an_i . bn_i for local rows [0, 2048)
            prod = projp.tile([128, 2048], BF16, tag="e1", name="prod")
            nc.vector.tensor_tensor(prod[:], anT[:, 0:R], bnT[:, 0:R], OP.mult)
            psNm = ps_tile()
            mm_fill(psNm[0:1, :], ones_col_bf[:], prod[:], 2048)
            nc.vector.tensor_copy(num_sb[:], psNm[0:1, :])
            nc.sync.dma_start(num_d[:], num_sb[:])

            drain_s11(len(s11_units) - s11_pos[0])

            # ------------------------- S12 full row block -------------------------
            for p in range(NPANEL):
                sl = slice(p * 2048, (p + 1) * 2048)
                colacc12 = cacc12p.tile([128, 2048], BF16, tag="cacc12", name="colacc12")
                for rl in range(NSTRIP):
                    lhsT = anT[:, rl * 128 : (rl + 1) * 128]
                    ps = ps_tile()
                    if rl == 0:
                        # first strip's exp lands directly in the column acc
                        mm_fill(ps, lhsT, bnT[:, sl], 2048)
                        nc.scalar.activation(
                            colacc12[:],
                            ps[:],
                            AF.Exp,
                            scale=INV_TAU,
                            accum_out=acc12[:, rl * 8 + p : rl * 8 + p + 1],
                        )
                    else:
                        mm_fill(ps, lhsT, bnT[:, sl], 2048)
                        E = epool.tile([128, 2048], BF16, tag="E", name="E")
                        nc.scalar.activation(
                            E[:],
                            ps[:],
                            AF.Exp,
                            scale=INV_TAU,
                            accum_out=acc12[:, rl * 8 + p : rl * 8 + p + 1],
                        )
                        nc.vector.tensor_tensor(colacc12[:], E[:], colacc12[:], OP.add)
                nc.sync.dma_start(cs12_d[:, sl], colacc12[:])

            for u in sym_band_units(bnT, acc22, cs22_d):
                u()

            # ------------------------- rowsum reduction -------------------------
            for rl in range(NSTRIP):
                nc.vector.tensor_reduce(
                    rs_sb[0][:, rl : rl + 1],
                    acc11[:, rl * 5 : (rl + 1) * 5],
                    mybir.AxisListType.X,
                    OP.add,
                )
                nc.vector.tensor_reduce(
                    rs_sb[1][:, rl : rl + 1],
                    acc12[:, rl * 8 : (rl + 1) * 8],
                    mybir.AxisListType.X,
                    OP.add,
                )
                nc.vector.tensor_reduce(
                    rs_sb[2][:, rl : rl + 1],
                    acc22[:, rl * 5 : (rl + 1) * 5],
                    mybir.AxisListType.X,
                    OP.add,
                )
            for i in range(3):
                nc.sync.dma_start(rs_d[i][:], rs_sb[i][:])

    return nc


def _get_nc():
    global _NC_CACHE
    if _NC_CACHE is None:
        _NC_CACHE = _build()
    return _NC_CACHE


def kernel(z1, z2, W1, b1, W2, b2):
    global LAST_RES
    bf = ml_dtypes.bfloat16
    z1 = np.asarray(z1, dtype=np.float32)
    z2 = np.asarray(z2, dtype=np.float32)
    W1 = np.asarray(W1, dtype=np.float32)
    W2 = np.asarray(W2, dtype=np.float32)
    b1 = np.asarray(b1, dtype=np.float32)
    b2 = np.asarray(b2, dtype=np.float32)
    # fold the "-1" of elu(y) = (min(exp y,1)+max(y,0)) - 1 into the 2nd bias
    b2p = (b2.astype(np.float64) - W2.astype(np.float64).sum(0)).astype(np.float32)

    nc = _get_nc()
    in_maps = []
    for c in range(NCORES):
        in_maps.append(
            {
                "z1": np.roll(z1, -c * R, axis=0).astype(bf),
                "z2": np.roll(z2, -c * R, axis=0).astype(bf),
                "w1": W1.astype(bf),
                "w2": W2.astype(bf),
                "b1": b1.reshape(D, 1).copy(),
                "b2p": b2p.reshape(D, 1).copy(),
            }
        )
    res = run_bass_kernel_spmd(nc, in_maps, list(range(NCORES)), **RUN_KWARGS)
    LAST_RES = res

    e2 = np.exp(np.float64(INV_TAU))
    rs11o = np.empty(N, np.float64)
    rs12o = np.empty(N, np.float64)
    rs22o = np.empty(N, np.float64)
    lognum = np.empty(N, np.float64)
    cs12 = np.zeros(N, np.float64)
    cs11 = np.zeros(N, np.float64)
    cs22 = np.zeros(N, np.float64)
    for c in range(NCORES):
        r = res.results[c]
        sl = slice(c * R, (c + 1) * R)
        rs11o[sl] = r["rs0"].astype(np.float64).T.reshape(R)
        rs12o[sl] = r["rs1"].astype(np.float64).T.reshape(R)
        rs22o[sl] = r["rs2"].astype(np.float64).T.reshape(R)
        lognum[sl] = r["numd"].astype(np.float64).reshape(R) * INV_TAU
        cs12 += np.roll(r["cs12"].astype(np.float64).sum(0), c * R)
        e11 = np.zeros(N, np.float64)
        e11[128 : 128 + CACC_W] = r["cs11"].astype(np.float64).sum(0)
        cs11 += np.roll(e11, c * R)
        e22 = np.zeros(N, np.float64)
        e22[128 : 128 + CACC_W] = r["cs22"].astype(np.float64).sum(0)
        cs22 += np.roll(e22, c * R)

    den1 = rs11o + cs11 + rs12o - e2
    den2 = rs22o + cs22 + cs12 - e2
    loss = np.mean(0.5 * (np.log(den1) + np.log(den2)) - lognum)
    return np.array(loss, dtype=np.float32)


# revision 30
# speedup vs baseline: 1.5390x; 1.0089x over previous
"""Trainium2 Bass kernel for the HGCA contrastive loss (nn_HGCA_10857677324785).

loss = mean over i of 0.5*(l1_i + l2_i) where
  h1 = elu(z1@W1+b1)@W2+b2 ; h2 likewise ; an, bn = l2-normalized rows
  l1_i = -log( exp(an_i.bn_i/tau) / (sum_j exp(an_i.an_j/tau)
               + sum_j exp(an_i.bn_j/tau) - e^{1/tau}) )
  l2_i symmetric with row sums of exp(bn@bn.T) and exp(bn@an.T).

Distribution: rows sharded over 8 cores (host rolls z1/z2 per core so the
core's rows sit at local rows [0,2048)). Each core computes the full
normalized projections, then:
  - S12 = an@bn.T row-block fully (rowsums via ACT accum, colsums via bf16
    column accumulators on DVE + PE ones-reduce)  [for l1 and l2]
  - S11/S22 exploit symmetry: each 128-row strip computes only the cyclic
    diagonal band d in [0,64] (d = col_block - row_block). Row sums at
    distance >= 65 are recovered from column sums of the transposed blocks
    computed by the mirror strips (on whatever core owns them). This saves
    ~1/3 of the exp work, the hard floor of this kernel (exp runs only on
    the scalar engine at 128 lanes/cycle).
Host assembles the scalar loss from O(N) partial sums in float64.
"""

import ml_dtypes
import numpy as np

import concourse.bass as bass
import concourse.tile as tile
from concourse import mybir
from concourse.bass_utils import run_bass_kernel_spmd

N = 16384
D = 128
NCORES = 8
R = N // NCORES          # 2048 rows per core
NSTRIP = R // 128        # 16 strips of 128 rows
NPANEL = N // 2048       # 8 col panels for S12
ND = 4                   # d-panels (d in [1,16],[17,32],[33,48],[49,64])
CACC_W = 79 * 128        # colacc11/22 width: col blocks 1..79
INV_TAU = 2.0
F32 = mybir.dt.float32
BF16 = mybir.dt.bfloat16
AF = mybir.ActivationFunctionType
OP = mybir.AluOpType

# This walrus build supports at most 2 sync waits per instruction; Tile's sem
# assignment freely emits 3-11. Post-pass: hoist excess waits onto injected
# same-engine EventSemaphore fillers (engine queues are FIFO, so waits on an
# earlier filler happen-before the original instruction executes).

_MAX_WAITS = 1


def _split_waits(nc):
    for fn in nc.m.functions:
        for bb in fn.blocks:
            insts = list(bb.instructions)
            out = []
            changed = False
            for inst in insts:
                si = inst.sync_info
                w = list(si.on_wait) if si and si.on_wait else []
                if len(w) > _MAX_WAITS:
                    changed = True
                    extra, keep = w[:-_MAX_WAITS], w[-_MAX_WAITS:]
                    for i in range(0, len(extra), _MAX_WAITS):
                        f = mybir.InstEventSemaphore(
                            name=f"{inst.name}_wsplit{i}",
                            engine=inst.engine,
                            ins=[],
                            outs=[],
                            sync_info=mybir.SyncInfo(
                                on_wait=extra[i : i + _MAX_WAITS], on_update=[]
                            ),
                        )
                        out.append(f)
                    inst.sync_info = mybir.SyncInfo(
                        on_wait=keep,
                        on_update=list(si.on_update) if si.on_update else [],
                    )
                out.append(inst)
            if changed:
                bb.instructions = out


from concourse.vector_clock import ScopedClock


def _patched_drain_and_barrier(self, tick_clock, wait_clock):
    nc = self.nc
    drain_inst = nc.sync.drain()
    wait_clock.add_sem_waits(
        drain_inst.ins, ScopedClock({None: tick_clock.global_clock})
    )
    nc.all_engine_barrier()
    assert self.sems is not None
    popped = nc._tile_sem_poison_stack.pop()
    assert popped is self._sem_poison
    nc.clear_and_free_semaphores(list(self.sems.allocated().values()))
    nc.all_engine_barrier()
    _split_waits(nc)


tile.TileContext._drain_and_barrier = _patched_drain_and_barrier

_NC_CACHE = None
RUN_KWARGS: dict = {}
LAST_RES = None


def _build():
    nc = bass.Bass("TRN2", target_bir_lowering=False, debug=False)

    z1_d = nc.dram_tensor("z1", [N, D], BF16, kind="ExternalInput").ap()
    z2_d = nc.dram_tensor("z2", [N, D], BF16, kind="ExternalInput").ap()
    w1_d = nc.dram_tensor("w1", [D, D], BF16, kind="ExternalInput").ap()
    w2_d = nc.dram_tensor("w2", [D, D], BF16, kind="ExternalInput").ap()
    b1_d = nc.dram_tensor("b1", [D, 1], F32, kind="ExternalInput").ap()
    b2p_d = nc.dram_tensor("b2p", [D, 1], F32, kind="ExternalInput").ap()

    rs_d = [
        nc.dram_tensor(f"rs{i}", [128, NSTRIP], F32, kind="ExternalOutput").ap()
        for i in range(3)
    ]  # 0: S11 own-band rowsums, 1: S12 rowsums, 2: S22 own-band rowsums
    cs12_d = nc.dram_tensor("cs12", [128, N], BF16, kind="ExternalOutput").ap()
    cs11_d = nc.dram_tensor("cs11", [128, CACC_W], BF16, kind="ExternalOutput").ap()
    cs22_d = nc.dram_tensor("cs22", [128, CACC_W], BF16, kind="ExternalOutput").ap()
    num_d = nc.dram_tensor("numd", [1, R], BF16, kind="ExternalOutput").ap()
    scrP_d = nc.dram_tensor("scrP", [16, 2048], F32, kind="Internal").ap()
    scrU_d = nc.dram_tensor("scrU", [16, 2048], BF16, kind="Internal").ap()

    with tile.TileContext(nc) as tc:
        with (
            tc.tile_pool(name="pers", bufs=1) as pers,
            tc.tile_pool(name="cacc", bufs=1) as caccp,
            tc.tile_pool(name="cacc12", bufs=2) as cacc12p,
            tc.tile_pool(name="consts", bufs=1) as consts,
            tc.tile_pool(name="zt", bufs=1) as ztp,
            tc.tile_pool(name="proj", bufs=2) as projp,
            tc.tile_pool(name="npk", bufs=1) as npkp,
            tc.tile_pool(name="nrows", bufs=2) as nrowp,
            tc.tile_pool(name="irows", bufs=3) as irowp,
            tc.tile_pool(name="epool", bufs=3) as epool,
            tc.tile_pool(name="pA", bufs=1, space="PSUM") as pA,
            tc.tile_pool(name="pB", bufs=1, space="PSUM") as pB,
        ):
            # ---------------- persistent / const tiles ----------------
            anT = pers.tile([128, N], BF16, tag="anT")
            bnT = pers.tile([128, N], BF16, tag="bnT")
            acc11 = pers.tile([128, NSTRIP * 5], F32, tag="acc11")
            acc22 = pers.tile([128, NSTRIP * 5], F32, tag="acc22")
            acc12 = pers.tile([128, NSTRIP * 8], F32, tag="acc12")
            rs_sb = [
                pers.tile([128, NSTRIP], F32, tag=f"rs{i}", name=f"rs_sb{i}")
                for i in range(3)
            ]
            num_sb = pers.tile([1, R], BF16, tag="numsb")
            # packed per-column squared norms / inverse norms:
            # nsqP[p, t*128 + 16k + q] = |h_t column (k*2048 + q*128 + p)|^2
            nsqP = npkp.tile([128, 256], F32, tag="nsqP")
            invP = npkp.tile([128, 256], BF16, tag="invP")

            ones_col_bf = consts.tile([128, 1], BF16, tag="ocb")
            nc.gpsimd.memset(ones_col_bf[:], 1.0)
            ones_row_bf = consts.tile([1, 128], BF16, tag="orb")
            nc.gpsimd.memset(ones_row_bf[:], 1.0)
            w1sb = consts.tile([128, 128], BF16, tag="w1")
            nc.sync.dma_start(w1sb[:], w1_d[:])
            w2sb = consts.tile([128, 128], BF16, tag="w2")
            nc.sync.dma_start(w2sb[:], w2_d[:])
            b1sb = consts.tile([128, 1], F32, tag="b1")
            nc.sync.dma_start(b1sb[:], b1_d[:])
            b2psb = consts.tile([128, 1], F32, tag="b2p")
            nc.sync.dma_start(b2psb[:], b2p_d[:])

            # PSUM [128,2048] alternation between the two 4-bank pools
            _ps_state = [0]

            def ps_tile():
                _ps_state[0] ^= 1
                pool = pA if _ps_state[0] else pB
                return pool.tile([128, 2048], F32, tag="mm", name="ps")

            def mm_fill(ps, lhsT, rhs_base, cols, ps_off=0):
                q = 0
                while q < cols:
                    w = min(512, cols - q)
                    nc.tensor.matmul(
                        ps[:, ps_off + q : ps_off + q + w],
                        lhsT,
                        rhs_base[:, q : q + w],
                    )
                    q += w

            # ---------------- projection ----------------
            def transpose_in(z_d):
                zT = ztp.tile([128, N], BF16, tag="zT", name="zT")
                bounds = [0, 2048, 4096, 8192, 12288, N]
                for i in range(len(bounds) - 1):
                    lo, hi = bounds[i], bounds[i + 1]
                    nc.sync.dma_start_transpose(zT[:, lo:hi], z_d[lo:hi, :])
                return zT

            def pass1_chunk(zT, hT, k, relu_act=False):
                """hT[:, chunk k] <- elu(zT @ W1 + b1) + 1"""
                sl = slice(k * 2048, (k + 1) * 2048)
                psA = ps_tile()
                mm_fill(psA, w1sb[:], zT[:, sl], 2048)
                e1 = projp.tile([128, 2048], BF16, tag="e1")
                nc.scalar.activation(e1[:], psA[:], AF.Exp, bias=b1sb[:])
                r1 = projp.tile([128, 2048], BF16, tag="r1")
                if relu_act:
                    nc.scalar.activation(r1[:], psA[:], AF.Relu, bias=b1sb[:])
                else:
                    nc.vector.tensor_scalar(r1[:], psA[:], b1sb[:], 0.0, OP.add, OP.max)
                nc.vector.scalar_tensor_tensor(
                    hT[:, sl], e1[:], 1.0, r1[:], OP.min, OP.add
                )

            invrows = {}

            def pass2_chunk(hT, t, k, nrow_act=True):
                """hT chunk k <- elu1 @ W2 + b2'; per-column inverse norms
                computed via a DRAM-bounced pack -> [128,16] ln/exp ->
                DRAM-bounced unpack into invrows[(t, k)]."""
                sl = slice(k * 2048, (k + 1) * 2048)
                psB = ps_tile()
                mm_fill(psB, w2sb[:], hT[:, sl], 2048)
                nc.scalar.activation(hT[:, sl], psB[:], AF.Identity, bias=b2psb[:])
                sq = projp.tile([128, 2048], BF16, tag="sq")
                nc.vector.tensor_tensor(sq[:], hT[:, sl], hT[:, sl], OP.mult)
                psN = ps_tile()
                mm_fill(psN[0:1, :], ones_col_bf[:], sq[:], 2048)
                nrow = nrowp.tile([1, 2048], F32, tag="nrow")
                if nrow_act:
                    nc.scalar.activation(nrow[:], psN[0:1, :], AF.Copy)
                else:
                    nc.vector.tensor_copy(nrow[:], psN[0:1, :])
                srow = t * 8 + k
                nc.sync.dma_start(scrP_d[srow : srow + 1, :], nrow[:])
                csl = slice(t * 128 + 16 * k, t * 128 + 16 * k + 16)
                nc.sync.dma_start(
                    nsqP[:, csl],
                    scrP_d[srow : srow + 1, :].rearrange("a (q p) -> (a p) q", p=128),
                )
                nc.scalar.activation(nsqP[:, csl], nsqP[:, csl], AF.Ln)
                nc.scalar.activation(invP[:, csl], nsqP[:, csl], AF.Exp, scale=-0.5)
                nc.sync.dma_start(
                    scrU_d[srow : srow + 1, :].rearrange("a (q p) -> (a p) q", p=128),
                    invP[:, csl],
                )
                invnb = irowp.tile([128, 2048], BF16, tag="invnb")
                nc.sync.dma_start(
                    invnb[:], scrU_d[srow : srow + 1, :].broadcast_to((128, 2048))
                )
                invrows[(t, k)] = invnb

            def apply_chunk(hT, t, k):
                sl = slice(k * 2048, (k + 1) * 2048)
                invnb = invrows.pop((t, k))
                nc.vector.tensor_tensor(hT[:, sl], hT[:, sl], invnb[:], OP.mult)

            # ------------- symmetric matrices: cyclic diagonal band -------------
            def sym_band_units(xT, acc, cs_dram):
                colacc = caccp.tile([128, CACC_W], BF16, tag="cacc", name="colacc")
                units = []

                def u_init():
                    nc.gpsimd.memset(colacc[:], 0.0)
                    psD = ps_tile()
                    for rl in range(NSTRIP):
                        nc.tensor.matmul(
                            psD[:, rl * 128 : (rl + 1) * 128],
                            xT[:, rl * 128 : (rl + 1) * 128],
                            xT[:, rl * 128 : (rl + 1) * 128],
                        )
                    eD = epool.tile([128, 2048], BF16, tag="E", name="eD")
                    nc.scalar.activation(eD[:], psD[:], AF.Exp, scale=INV_TAU)
                    for rl in range(NSTRIP):
                        nc.vector.tensor_reduce(
                            acc[:, rl * 5 + 4 : rl * 5 + 5],
                            eD[:, rl * 128 : (rl + 1) * 128],
                            mybir.AxisListType.X,
                            OP.add,
                        )

                units.append(u_init)

                def u_strip(dp, rl):
                    lhsT = xT[:, rl * 128 : (rl + 1) * 128]
                    c0 = (rl + 1 + 16 * dp) * 128
                    ps = ps_tile()
                    mm_fill(ps, lhsT, xT[:, c0 : c0 + 2048], 2048)
                    E = epool.tile([128, 2048], BF16, tag="E", name="E")
                    nc.scalar.activation(
                        E[:],
                        ps[:],
                        AF.Exp,
                        scale=INV_TAU,
                        accum_out=acc[:, rl * 5 + dp : rl * 5 + dp + 1],
                    )
                    # colacc excludes d=64 (the last block of dp==3)
                    w = 1920 if dp == ND - 1 else 2048
                    t0 = (rl + 16 * dp) * 128
                    nc.vector.tensor_tensor(
                        colacc[:, t0 : t0 + w],
                        E[:, 0:w],
                        colacc[:, t0 : t0 + w],
                        OP.add,
                    )

                for dp in range(ND):
                    for rl in range(NSTRIP):
                        units.append(lambda dp=dp, rl=rl: u_strip(dp, rl))

                def u_out():
                    nc.sync.dma_start(cs_dram[:, :], colacc[:])

                units.append(u_out)
                return units

            # ---- schedule ----
            s11_units = None
            s11_pos = [0]

            def drain_s11(n):
                for _ in range(n):
                    if s11_pos[0] < len(s11_units):
                        s11_units[s11_pos[0]]()
                        s11_pos[0] += 1

            zT1 = transpose_in(z1_d)
            for k in range(8):
                pass1_chunk(zT1, anT, k, relu_act=True)
            zT2 = transpose_in(z2_d)  # DMA overlaps z1 PASS2 (zT1 dead)

            # fused: z1 PASS2+normalize with z2 PASS1 (independent tensors)
            for k in range(8):
                pass2_chunk(anT, 0, k)
                pass1_chunk(zT2, bnT, k)
                if k >= 2:
                    apply_chunk(anT, 0, k - 2)
                if k == 3:
                    s11_units = sym_band_units(anT, acc11, cs11_d)
                if k >= 4:
                    drain_s11(2)
            apply_chunk(anT, 0, 6)
            apply_chunk(anT, 0, 7)
            drain_s11(2)

            for k in range(8):
                pass2_chunk(bnT, 1, k, nrow_act=False)
                if k >= 2:
                    apply_chunk(bnT, 1, k - 2)
                drain_s11(3)
            apply_chunk(bnT, 1, 6)
            apply_chunk(bnT, 1, 7)

            # num: diagonal dots an_i . bn_i for local rows [0, 2048)
            prod = projp.tile([128, 2048], BF16, tag="e1", name="prod")
            nc.vector.tensor_tensor(prod[:], anT[:, 0:R], bnT[:, 0:R], OP.mult)
            psNm = ps_tile()
            mm_fill(psNm[0:1, :], ones_col_bf[:], prod[:], 2048)
            nc.vector.tensor_copy(num_sb[:], psNm[0:1, :])
            nc.sync.dma_start(num_d[:], num_sb[:])

            drain_s11(len(s11_units) - s11_pos[0])

            # ------------------------- S12 full row block -------------------------
            for p in range(NPANEL):
                sl = slice(p * 2048, (p + 1) * 2048)
                colacc12 = cacc12p.tile([128, 2048], BF16, tag="cacc12", name="colacc12")
                for rl in range(NSTRIP):
                    lhsT = anT[:, rl * 128 : (rl + 1) * 128]
                    ps = ps_tile()
                    if rl == 0:
                        # first strip's exp lands directly in the column acc
                        mm_fill(ps, lhsT, bnT[:, sl], 2048)
                        nc.scalar.activation(
                            colacc12[:],
                            ps[:],
                            AF.Exp,
                            scale=INV_TAU,
                            accum_out=acc12[:, rl * 8 + p : rl * 8 + p + 1],
                        )
                    else:
                        mm_fill(ps, lhsT, bnT[:, sl], 2048)
                        E = epool.tile([128, 2048], BF16, tag="E", name="E")
                        nc.scalar.activation(
                            E[:],
                            ps[:],
                            AF.Exp,
                            scale=INV_TAU,
                            accum_out=acc12[:, rl * 8 + p : rl * 8 + p + 1],
                        )
                        nc.vector.tensor_tensor(colacc12[:], E[:], colacc12[:], OP.add)
                nc.sync.dma_start(cs12_d[:, sl], colacc12[:])

            for u in sym_band_units(bnT, acc22, cs22_d):
                u()

            # ------------------------- rowsum reduction -------------------------
            for rl in range(NSTRIP):
                nc.vector.tensor_reduce(
                    rs_sb[0][:, rl : rl + 1],
                    acc11[:, rl * 5 : (rl + 1) * 5],
                    mybir.AxisListType.X,
                    OP.add,
                )
                nc.vector.tensor_reduce(
                    rs_sb[1][:, rl : rl + 1],
                    acc12[:, rl * 8 : (rl + 1) * 8],
                    mybir.AxisListType.X,
                    OP.add,
                )
                nc.vector.tensor_reduce(
                    rs_sb[2][:, rl : rl + 1],
                    acc22[:, rl * 5 : (rl + 1) * 5],
                    mybir.AxisListType.X,
                    OP.add,
                )
            for i in range(3):
                nc.sync.dma_start(rs_d[i][:], rs_sb[i][:])

    return nc


def _get_nc():
    global _NC_CACHE
    if _NC_CACHE is None:
        _NC_CACHE = _build()
    return _NC_CACHE


def kernel(z1, z2, W1, b1, W2, b2):
    global LAST_RES
    bf = ml_dtypes.bfloat16
    z1 = np.asarray(z1, dtype=np.float32)
    z2 = np.asarray(z2, dtype=np.float32)
    W1 = np.asarray(W1, dtype=np.float32)
    W2 = np.asarray(W2, dtype=np.float32)
    b1 = np.asarray(b1, dtype=np.float32)
    b2 = np.asarray(b2, dtype=np.float32)
    # fold the "-1" of elu(y) = (min(exp y,1)+max(y,0)) - 1 into the 2nd bias
    b2p = (b2.astype(np.float64) - W2.astype(np.float64).sum(0)).astype(np.float32)

    nc = _get_nc()
    in_maps = []
    for c in range(NCORES):
        in_maps.append(
            {
                "z1": np.roll(z1, -c * R, axis=0).astype(bf),
                "z2": np.roll(z2, -c * R, axis=0).astype(bf),
                "w1": W1.astype(bf),
                "w2": W2.astype(bf),
                "b1": b1.reshape(D, 1).copy(),
                "b2p": b2p.reshape(D, 1).copy(),
            }
        )
    res = run_bass_kernel_spmd(nc, in_maps, list(range(NCORES)), **RUN_KWARGS)
    LAST_RES = res

    e2 = np.exp(np.float64(INV_TAU))
    rs11o = np.empty(N, np.float64)
    rs12o = np.empty(N, np.float64)
    rs22o = np.empty(N, np.float64)
    lognum = np.empty(N, np.float64)
    cs12 = np.zeros(N, np.float64)
    cs11 = np.zeros(N, np.float64)
    cs22 = np.zeros(N, np.float64)
    for c in range(NCORES):
        r = res.results[c]
        sl = slice(c * R, (c + 1) * R)
        rs11o[sl] = r["rs0"].astype(np.float64).T.reshape(R)
        rs12o[sl] = r["rs1"].astype(np.float64).T.reshape(R)
        rs22o[sl] = r["rs2"].astype(np.float64).T.reshape(R)
        lognum[sl] = r["numd"].astype(np.float64).reshape(R) * INV_TAU
        cs12 += np.roll(r["cs12"].astype(np.float64).sum(0), c * R)
        e11 = np.zeros(N, np.float64)
        e11[128 : 128 + CACC_W] = r["cs11"].astype(np.float64).sum(0)
        cs11 += np.roll(e11, c * R)
        e22 = np.zeros(N, np.float64)
        e22[128 : 128 + CACC_W] = r["cs22"].astype(np.float64).sum(0)
        cs22 += np.roll(e22, c * R)

    den1 = rs11o + cs11 + rs12o - e2
    den2 = rs22o + cs22 + cs12 - e2
    loss = np.mean(0.5 * (np.log(den1) + np.log(den2)) - lognum)
    return np.array(loss, dtype=np.float32)
